# revision 4
# baseline (speedup 1.0000x reference)
"""MixProp GNN kernel for 8x Trainium2 NeuronCores — fp8 DoubleRow version.

Math (per batch b, with X = x[b] as [N, C*T] node-major):
    A    = (adj + I) / deg[None, :]          (column-normalized, numpy)
    P1   = A @ X,  P2 = A @ P1               (adjacency powers, on device)
    y    = sigmoid(V0 @ X + V1 @ P1 + V2 @ P2 + bias)
with the MixProp alpha-mixing folded into the projection weights:
    V0 = W0 + a*W1 + a*W2,  V1 = W1 + a*W2,  V2 = W2.

Speed plan (cost-model driven):
  * Propagation matmuls run in fp8e4 with MatmulPerfMode.DoubleRow:
    256 contraction rows per instruction at 0.5 PE cycles/row — 2x bf16.
    A is pre-scaled by 2^17 on the host so its ~5e-4 entries sit in the
    fp8e4 normal range; each propagation output is rescaled by 2^-11 on
    the psum->sbuf copy, so the on-chip streams stay O(1) (64*P1, 64*P2)
    and the 1/64 factors fold into the projection weights.
  * P1 feeds step 2 straight from SBUF (fp8 cast on the Act engine) —
    no DRAM roundtrip for the natural-order activation.
  * Channel-major slabs for the projection come from the XBAR DMA
    transpose (dma_start_transpose, 14ns per 16x128 tile): with free
    layout f = t_hi*128 + (c*4 + t_lo), transposing a [n, f] stage tile
    yields slab[j=(c,t_lo), q=t_hi, n] directly.
  * Projection contracts over the (c, t_lo) partition dim with
    block-diagonal stationaries V_k (x) I_4 ([128, 128]), accumulating
    three matmuls (X, 64*P1, 4096*P2 slabs) into a [128, 512] psum;
    sigmoid+bias fused on Act; y written bf16 and decoded on the host.
  * Scheduling: panel halves split across the SWDGE and SP DMA queues
    (SWDGE's 128-descriptor in-flight window serializes one queue),
    projection software-pipelined 1.5 blocks behind propagation, step-1
    transposes issued one block late so panel loads lead the SP queue.

Sharding: data-parallel over batch B=8, one batch element per core.
"""

import numpy as np

B, C, N, T = 8, 32, 4096, 32
ALPHA = 0.05
C_OUT = 32
CT = C * T            # 1024
NT = N * T            # 131072
P = 128               # SBUF partitions
NV = N // P           # 32 output row tiles
NW = N // P           # 32 contraction chunks
FS = 512              # psum free-dim slice
TL = 4                # t_lo values packed with c into the partition dim
TH = T // TL          # 8 t_hi blocks
A_SCALE = 2.0 ** 17   # host pre-scale on A (fp8 range)
S_SCALE = 2.0 ** -11  # psum->sbuf copy scale; 2^17 * 2^-11 = 64


def _build_nc():
    import concourse.mybir as mybir
    from concourse import bacc
    from concourse.tile import TileContext

    F32 = mybir.dt.float32
    BF16 = mybir.dt.bfloat16
    FP8 = mybir.dt.float8e4
    DR = mybir.MatmulPerfMode.DoubleRow

    nc = bacc.Bacc(num_swdge_queues=4)

    # a8[vt, p, wc, v] = fp8(2^17 * A^T)[wc*128+p, vt*128+v]
    a8_d = nc.dram_tensor("a8", [NV, P, NW, P], FP8, kind="ExternalInput")
    # x8[p, wc, f] = fp8(X)[wc*128+p, f], f = t_hi*128 + c*4 + t_lo
    x8_d = nc.dram_tensor("x8", [P, NW, CT], FP8, kind="ExternalInput")
    # xq[(c,t_lo), (vt, t_hi, n)] = bf16(X) slab-major
    xq_d = nc.dram_tensor("xq", [P, NV * TH * P], BF16, kind="ExternalInput")
    # block-diagonal stationaries [V0 (x) I4; V1/64 (x) I4; V2/64 (x) I4]
    vst_d = nc.dram_tensor("vst", [P, 3, P], BF16, kind="ExternalInput")
    bias_d = nc.dram_tensor("bias", [P, 1], F32, kind="ExternalInput")
    # y[(o,t_lo), (vt, t_hi, n)]
    y_d = nc.dram_tensor("y", [P, NV * TH * P], BF16, kind="ExternalOutput")

    with TileContext(nc) as tc:
        with (
            tc.tile_pool(name="resid", bufs=1) as resid_pool,
            tc.tile_pool(name="panel", bufs=3) as panel_pool,
            tc.tile_pool(name="stage", bufs=4) as stage_pool,
            tc.tile_pool(name="xs", bufs=3) as xs_pool,
            tc.tile_pool(name="s2", bufs=4) as s2_pool,
            tc.tile_pool(name="yout", bufs=2) as yout_pool,
            tc.tile_pool(name="consts", bufs=1) as const_pool,
            tc.tile_pool(name="psum_p", bufs=4, space="PSUM") as psum_pool,
            tc.tile_pool(name="psum_y", bufs=2, space="PSUM") as psum_y_pool,
        ):
            vst_t = const_pool.tile([P, 3, P], BF16, tag="vst")
            nc.sync.dma_start(vst_t, vst_d[:, :, :])
            bias_t = const_pool.tile([P, 1], F32, tag="bias")
            nc.sync.dma_start(bias_t, bias_d[:, :])

            x8_t = resid_pool.tile([P, NW, CT], FP8, tag="x8")
            r1_t = resid_pool.tile([P, NW, CT], FP8, tag="r1")
            # SBUF-resident slab-major 64*P1 (written by step-1 transposes)
            s1q_t = resid_pool.tile([P, NV * TH * P], BF16, tag="s1q")

            def load_panel_pair(vh):
                pp = panel_pool.tile([P, 2, NW, P], FP8, tag="panel")
                for h, eng in ((0, nc.gpsimd), (1, nc.sync)):
                    # split halves across the SWDGE and SP queues: the SWDGE
                    # 128-descriptor in-flight window serializes its DMAs, so
                    # one queue alone can't stream panels with slack
                    eng.dma_start(
                        pp[:, h:h + 1, :, :],
                        a8_d[2 * vh + h:2 * vh + h + 1, :, :, :].rearrange(
                            "h p w v -> p h w v"
                        ),
                    )
                return pp

            def propagate(panel, h, rhs, stage, extra_psum=False):
                # one [128, 1024] output tile of (2^17*A) @ rhs, scaled
                # into bf16 stage (at free offset h*CT) via DVE. In step 1
                # the (otherwise idle) psum_y pool doubles the psum ring.
                for fi in range(CT // FS):
                    pool = psum_y_pool if (extra_psum and fi == 1) else psum_pool
                    ps = pool.tile([P, FS], F32, tag="ps")
                    for wp in range(NW // 2):
                        nc.tensor.matmul(
                            ps,
                            panel[:, 2 * wp:2 * wp + 2, :],
                            rhs[:, 2 * wp:2 * wp + 2, fi * FS:(fi + 1) * FS],
                            start=(wp == 0),
                            stop=(wp == NW // 2 - 1),
                            perf_mode=DR,
                        )
                    if fi == 0:
                        nc.vector.tensor_scalar_mul(
                            stage[:, h * CT + fi * FS:h * CT + (fi + 1) * FS],
                            ps,
                            S_SCALE,
                        )
                    else:
                        nc.scalar.mul(
                            stage[:, h * CT + fi * FS:h * CT + (fi + 1) * FS],
                            ps,
                            S_SCALE,
                        )

            BLK = TH * P          # 1024 slab elements per vt

            # ---- step 1: 64*P1 = 2^-11 * (2^17 A) @ X ----
            # first block's panel half + a small x8 chunk lead the DMA
            # queue so the first psum chain starts ~3us in; panels prefetch
            # 2 blocks ahead of use thereafter
            p01 = panel_pool.tile([P, 2, NW, P], FP8, tag="panel")
            ph0 = p01[:, 0, :, :]
            ph1 = p01[:, 1, :, :]
            nc.sync.dma_start(
                p01[:, 0:1, :, :],
                a8_d[0:1, :, :, :].rearrange("h p w v -> p h w v"),
            )
            nc.sync.dma_start(x8_t[:, 0:2, :], x8_d[:, 0:2, :])
            nc.sync.dma_start(
                p01[:, 1:2, :, :],
                a8_d[1:2, :, :, :].rearrange("h p w v -> p h w v"),
            )
            nc.sync.dma_start(x8_t[:, 2:8, :], x8_d[:, 2:8, :])
            for ck in range(1, 4):
                nc.sync.dma_start(
                    x8_t[:, 8 * ck:8 * (ck + 1), :],
                    x8_d[:, 8 * ck:8 * (ck + 1), :],
                )
            panels = {1: load_panel_pair(1), 2: load_panel_pair(2)}
            prev_t = None
            for vh in range(NV // 2):
                stage = stage_pool.tile([P, 2 * CT], BF16, tag="stage")
                pp = panels.pop(vh) if vh > 0 else None
                if vh + 3 < NV // 2:
                    panels[vh + 3] = load_panel_pair(vh + 3)
                if prev_t is not None:
                    # previous block's transpose issued AFTER this block's
                    # panel load: panels never queue behind a stage wait
                    nc.sync.dma_start_transpose(*prev_t)
                for h in range(2):
                    vt = 2 * vh + h
                    if vh == 0:
                        panel = ph0 if h == 0 else ph1
                    else:
                        panel = pp[:, h, :, :]
                    propagate(panel, h, x8_t, stage, extra_psum=True)
                    # fp8 copy feeds step 2 (Act engine)
                    nc.scalar.copy(
                        r1_t[:, vt, :], stage[:, h * CT:(h + 1) * CT]
                    )
                prev_t = (
                    s1q_t[:, vh * 2 * BLK:(vh + 1) * 2 * BLK].rearrange(
                        "j (q n) -> j q n", n=P
                    ),
                    stage,
                )
            nc.sync.dma_start_transpose(*prev_t)

            # ---- step 2: 64*P2 tiles + fused projection (lagged 1 block) ----
            def project(blk):
                vh, xs, s2c = blk
                lo = vh * 2 * BLK
                yo = yout_pool.tile([P, 2 * BLK], BF16, tag="yo")
                for q in range(2 * BLK // FS):           # 4 psum_y tiles
                    psy = psum_y_pool.tile([P, FS], F32, tag="psy")
                    sl = slice(q * FS, (q + 1) * FS)
                    s1sl = slice(lo + q * FS, lo + (q + 1) * FS)
                    for i, rhs_ap in enumerate(
                        (xs[:, sl], s1q_t[:, s1sl], s2c[:, sl])
                    ):
                        nc.tensor.matmul(
                            psy,
                            vst_t[:, i, :],
                            rhs_ap,
                            start=(i == 0),
                            stop=(i == 2),
                            skip_group_check=True,
                        )
                    nc.scalar.activation(
                        yo[:, sl],
                        psy,
                        mybir.ActivationFunctionType.Sigmoid,
                        bias=bias_t,
                    )
                nc.sync.dma_start(y_d[:, lo:lo + 2 * BLK], yo)

            from collections import deque
            pending = deque()
            prev_t = None
            panels = {vh: load_panel_pair(vh) for vh in range(3)}
            for vh in range(NV // 2):
                lo = vh * 2 * BLK
                xs = xs_pool.tile([P, 2 * BLK], BF16, tag="xs")
                nc.sync.dma_start(xs, xq_d[:, lo:lo + 2 * BLK])
                s2c = s2_pool.tile([P, 2 * BLK], BF16, tag="s2c")
                stage = stage_pool.tile([P, 2 * CT], BF16, tag="stage")
                last = vh == NV // 2 - 1
                pp = panels.pop(vh)
                if vh + 3 < NV // 2:
                    panels[vh + 3] = load_panel_pair(vh + 3)
                for h in range(2):
                    propagate(pp[:, h, :, :], h, r1_t, stage)
                    if (h == 1 and len(pending) >= 2) or last:
                        project(pending.popleft())
                nc.sync.dma_start_transpose(
                    s2c.rearrange("j (q n) -> j q n", n=P),
                    stage,
                )
                pending.append((vh, xs, s2c))
            while pending:
                project(pending.popleft())

    nc.compile()
    return nc


def kernel(x, adj, w, b):
    return _run(x, adj, w, b)[0]


def _run(x, adj, w, b, trace=False, trace_kwargs=None):
    import ml_dtypes
    from concourse.bass_utils import run_bass_kernel_spmd

    FP8NP = ml_dtypes.float8_e4m3
    BF16NP = ml_dtypes.bfloat16
    x = np.ascontiguousarray(x, dtype=np.float32)
    adj = np.asarray(adj, dtype=np.float32)
    w = np.asarray(w, dtype=np.float32)
    b = np.asarray(b, dtype=np.float32)

    # Column-normalized adjacency with self loops, transposed for the PE,
    # pre-scaled into the fp8e4 normal range and tiled [vt, p, wc, v].
    adjp = adj + np.eye(N, dtype=np.float32)
    deg = adjp.sum(axis=1)
    at = np.ascontiguousarray(adjp.T) / deg[:, None]   # at[w, v] = A[v, w]
    a8 = (at * A_SCALE).astype(FP8NP)
    a8 = np.ascontiguousarray(
        a8.reshape(NW, P, NV, P).transpose(2, 1, 0, 3)
    )

    # Fold alpha-mixing and the 2^17 * 2^-11 = 64 stream scale into the
    # projection weights; expand each V_k to the block-diagonal V_k (x) I4
    # acting on (c, t_lo) partitions.
    w0, w1, w2 = w[:, 0:C], w[:, C:2 * C], w[:, 2 * C:3 * C]
    v0 = w0 + ALPHA * w1 + ALPHA * w2
    v1 = (w1 + ALPHA * w2) / 64.0
    v2 = w2 / 4096.0          # step-2 stream carries 2^17 * 2^-11 twice
    eye4 = np.eye(TL, dtype=np.float32)
    vst = np.stack(
        [
            np.einsum("oc,ab->caob", vk, eye4).reshape(P, P)
            for vk in (v0, v1, v2)
        ],
        axis=1,
    )                                                  # [128, 3, 128]
    vst = np.ascontiguousarray(vst.astype(BF16NP))
    bias = np.ascontiguousarray(
        np.repeat(b, TL).reshape(P, 1), dtype=np.float32
    )                                                  # bias[(o,t_lo)]

    nc = _build_nc()

    in_maps = []
    for bi in range(B):
        xb = x[bi]                                     # [C, N, T]
        # propagation free layout: f = t_hi*128 + c*4 + t_lo
        xf = xb.reshape(C, N, TH, TL).transpose(1, 2, 0, 3)   # [n,th,c,tl]
        x8 = np.ascontiguousarray(
            xf.reshape(N, CT).astype(FP8NP).reshape(NW, P, CT)
            .transpose(1, 0, 2)
        )                                              # [p, wc, f]
        # slab layout: [(c,t_lo), (vt, t_hi, n)]
        xq = np.ascontiguousarray(
            xb.reshape(C, NV, P, TH, TL).transpose(0, 4, 1, 3, 2)
            .reshape(P, NV * TH * P).astype(BF16NP)
        )
        in_maps.append(
            {"a8": a8, "x8": x8, "xq": xq, "vst": vst, "bias": bias}
        )

    kwargs = dict(trace_kwargs or {})
    res = run_bass_kernel_spmd(
        nc, in_maps, core_ids=list(range(B)), trace=trace, **kwargs
    )
    # y[(o,t_lo), (vt, t_hi, n)] -> [C_OUT, N, T]
    y = np.stack(
        [
            r["y"].astype(np.float32)
            .reshape(C_OUT, TL, NV, TH, P)
            .transpose(0, 2, 4, 3, 1)
            .reshape(C_OUT, N, T)
            for r in res.results
        ],
        axis=0,
    )
    return y, res

